# revision 6
# baseline (speedup 1.0000x reference)
"""Trainium2 Bass kernel for the controlled-unitary problem.

reference semantics (control=0, num_qubits=13, dim=8192):
    mask bit = 1 << 12, so columns/rows with that bit set are idx 4096..8191.
    out[:, c0] = state[:, c0]                       (control bit off: untouched)
    out[:, c1] = state[:, c1] @ target[c1, c1]      (controlled unitary)

Device work: complex [256,4096] @ [4096,4096] GEMM, Gauss 3-mult form.
Sharding: output columns of the GEMM split 8 ways (each core gets a
[4096, 512] slab of the target block; every weight byte moves once).

Per-core kernel (v3):
  - ONE packed input dram tensor x[128, KT, 2, 768] holding all four
    planes (a_r|b_r / a_i|b_i interleaved per k-tile) -> one dma_start
    per chunk, 3KB-per-partition descriptors, rings alternated.
  - Gauss prep on views: Vector computes a_s and negates a_i in place;
    GpSimd computes b_s and b_d (in place over b_i).
  - k1 matmuls first per chunk so the PE starts on a_s/b_r while the
    B-side preps still run; last chunk is m-major so m0's epilogue
    hides under m1's matmuls.
  - Epilogue adds read the two PSUM banks directly (one tensor_tensor
    per output plane), Vector and GpSimd in parallel, outputs DMA'd
    per m-tile on both rings.
"""

import os

import numpy as np

BATCH = 256
DIM = 8192
HALF = 4096
N_CORES = 8
NSH = HALF // N_CORES  # 512 output columns per core
KT = HALF // 128  # 32 k-tiles
MT = BATCH // 128  # 2 m-tiles
FW = BATCH + NSH  # 768 packed free width per (ktile, re/im)
CHUNKS = [1, 1, 2, 4, 8, 8, 8]  # k-tiles per DMA chunk (sums to KT)

DT_NAME = os.environ.get("KERNEL_DT", "float16")

_CACHE = {}


def _np_dtype(dt_name):
    if dt_name == "float16":
        return np.float16
    if dt_name == "bfloat16":
        import ml_dtypes

        return ml_dtypes.bfloat16
    return np.float32


def _build(dt_name):
    import concourse.mybir as mybir
    import concourse.tile as tile
    from concourse import bacc

    DT = getattr(mybir.dt, dt_name)
    F32 = mybir.dt.float32

    nc = bacc.Bacc("TRN2", target_bir_lowering=False, debug=False,
                   num_devices=N_CORES)

    x = nc.dram_tensor("x", [128, KT, 2, FW], DT, kind="ExternalInput")
    c = nc.dram_tensor("c", [2, BATCH, NSH], F32, kind="ExternalOutput")

    with tile.TileContext(nc) as tc:
        with (
            tc.tile_pool(name="xp", bufs=2) as xp,
            tc.tile_pool(name="dp", bufs=2) as dp,
            tc.tile_pool(name="op", bufs=1) as op,
            tc.tile_pool(name="ps", bufs=1, space="PSUM") as ps_pool,
        ):
            # Gauss 3-multiplication complex GEMM:
            #   k1 = (a_r+a_i).b_r   k2 = a_r.(b_i-b_r)   k3 = (-a_i).(b_r+b_i)
            #   C_r = k1 + k3        C_i = k1 + k2
            ps = {}
            for m in range(MT):
                for comp in ("k1", "k2", "k3"):
                    ps[(m, comp)] = ps_pool.tile(
                        [128, NSH], F32, name=f"ps_{m}_{comp}"
                    )

            rings = [nc.sync, nc.scalar]

            # PE warm-up: the HAM throttle holds the PE at 1.2 GHz for its
            # first ~3.4us of busy time.  The PE would otherwise idle while
            # the first DMA lands, so burn that window on dummy matmuls.
            warm = op.tile([128, NSH], DT, name="warm")
            wps = ps_pool.tile([128, NSH], F32, name="ps_warm")
            nc.gpsimd.memset(warm[:], 0)
            for _ in range(8):
                nc.tensor.matmul(wps[:], warm[:, :128], warm[:],
                                 start=True, stop=True)

            k0 = 0
            for ci, ch in enumerate(CHUNKS):
                nb = 3 if ch == 8 else 2
                x_t = xp.tile([128, ch, 2, FW], DT, name=f"x{ch}", bufs=nb)
                rings[ci % 2].dma_start(x_t[:], x[:, k0:k0 + ch, :, :])

                ar = x_t[:, :, 0, :BATCH]
                br = x_t[:, :, 0, BATCH:]
                ai = x_t[:, :, 1, :BATCH]
                bi = x_t[:, :, 1, BATCH:]
                as_t = dp.tile([128, ch, BATCH], DT, name=f"as{ch}", bufs=nb)
                bs_t = dp.tile([128, ch, NSH], DT, name=f"bs{ch}", bufs=nb)
                # prep: Vector handles the A side, GpSimd the B side
                nc.vector.tensor_tensor(as_t[:], ar, ai, mybir.AluOpType.add)
                nc.vector.tensor_scalar_mul(ai, ai, -1.0)  # a_n in place
                nc.gpsimd.tensor_tensor(bs_t[:], br, bi, mybir.AluOpType.add)
                nc.gpsimd.tensor_tensor(bi, bi, br,
                                        mybir.AluOpType.subtract)  # b_d

                operands = {
                    "k1": (as_t[:], br),
                    "k2": (ar, bi),
                    "k3": (ai, bs_t[:]),
                }
                last_chunk = k0 + ch == KT
                if last_chunk:
                    # m-major, k1 last per m: m0's epilogue hides under
                    # m1's matmuls, and the ACT copies of k2/k3 PSUM run
                    # while k1 is still streaming.
                    order = [(m, comp) for m in range(MT)
                             for comp in ("k2", "k3", "k1")]
                else:
                    order = [(m, comp) for comp in ("k1", "k2", "k3")
                             for m in range(MT)]
                for m, comp in order:
                    lhs_v, rhs_v = operands[comp]
                    msl = slice(m * 128, (m + 1) * 128)
                    for kk in range(ch):
                        k = k0 + kk
                        nc.tensor.matmul(
                            ps[(m, comp)][:], lhs_v[:, kk, msl],
                            rhs_v[:, kk, :], start=(k == 0),
                            stop=(last_chunk and kk == ch - 1),
                        )
                    if last_chunk and comp in ("k2", "k3"):
                        t = op.tile([128, NSH], F32, name=f"t_{comp}_{m}")
                        nc.scalar.activation(t[:], ps[(m, comp)][:],
                                             mybir.ActivationFunctionType.Copy)
                        if comp == "k2":
                            t2 = t
                        else:
                            t3 = t
                    if last_chunk and comp == "k1":
                        t1 = op.tile([128, NSH], F32, name=f"t_k1_{m}")
                        nc.scalar.activation(t1[:], ps[(m, "k1")][:],
                                             mybir.ActivationFunctionType.Copy)
                        out_r = op.tile([128, NSH], F32, name=f"or{m}")
                        out_i = op.tile([128, NSH], F32, name=f"oi{m}")
                        nc.vector.tensor_tensor(
                            out_r[:], t1[:], t3[:], mybir.AluOpType.add)
                        nc.gpsimd.tensor_tensor(
                            out_i[:], t1[:], t2[:], mybir.AluOpType.add)
                        rings[m % 2].dma_start(c[0, msl, :], out_r[:])
                        rings[(m + 1) % 2].dma_start(c[1, msl, :], out_i[:])
                k0 += ch

    nc.compile()
    return nc


def _get_nc(dt_name):
    if dt_name not in _CACHE:
        _CACHE[dt_name] = _build(dt_name)
    return _CACHE[dt_name]


def _pack_inputs(A, B, np_dt):
    """A: [256, 4096] complex64, B: [4096, 4096] complex64 (full slab).
    Returns per-core packed x arrays [128, KT, 2, 768]."""
    at = A.T  # [4096, 256]
    # [4096, F] -> [128, KT, F] with k = kt*128 + p
    def kxm(m):
        f = m.shape[1]
        return m.reshape(KT, 128, f).transpose(1, 0, 2)

    a_r = kxm(np.ascontiguousarray(at.real))
    a_i = kxm(np.ascontiguousarray(at.imag))
    xs = []
    for cidx in range(N_CORES):
        csl = slice(cidx * NSH, (cidx + 1) * NSH)
        b_r = kxm(np.ascontiguousarray(B.real[:, csl]))
        b_i = kxm(np.ascontiguousarray(B.imag[:, csl]))
        xc = np.empty((128, KT, 2, FW), dtype=np_dt)
        xc[:, :, 0, :BATCH] = a_r
        xc[:, :, 0, BATCH:] = b_r
        xc[:, :, 1, :BATCH] = a_i
        xc[:, :, 1, BATCH:] = b_i
        xs.append(xc)
    return xs


def run_device(A, B, dt_name=DT_NAME, trace=False):
    """A: [256, 4096] complex64, B: [4096, 4096] complex64.
    Returns C = A @ B as [256, 4096] complex64 plus the raw results."""
    from concourse import bass_utils

    nc = _get_nc(dt_name)
    np_dt = _np_dtype(dt_name)

    xs = _pack_inputs(A, B, np_dt)
    in_maps = [{"x": xc} for xc in xs]

    res = bass_utils.run_bass_kernel_spmd(
        nc, in_maps, core_ids=list(range(N_CORES)), trace=trace
    )

    out = np.empty((BATCH, HALF), dtype=np.complex64)
    for cidx in range(N_CORES):
        csl = slice(cidx * NSH, (cidx + 1) * NSH)
        out.real[:, csl] = res.results[cidx]["c"][0]
        out.imag[:, csl] = res.results[cidx]["c"][1]
    return out, res


def kernel(state, target_matrix, control, num_qubits):
    state = np.asarray(state)
    target_matrix = np.asarray(target_matrix)
    control = int(control)
    num_qubits = int(num_qubits)
    dim = 1 << num_qubits

    assert state.shape == (BATCH, DIM) and dim == DIM, (
        "kernel hardcoded for [256, 8192]"
    )

    mask = 1 << (num_qubits - control - 1)
    idx = np.arange(dim)
    c1 = idx[(idx & mask) != 0]  # columns with control bit set

    if control == 0:
        A = state[:, HALF:]
        B = target_matrix[HALF:, HALF:]
    else:
        A = state[:, c1]
        B = target_matrix[np.ix_(c1, c1)]
    A = np.ascontiguousarray(A, dtype=np.complex64)
    B = np.ascontiguousarray(B, dtype=np.complex64)

    C, _ = run_device(A, B)

    out = state.astype(np.complex64, copy=True)
    out[:, c1] = C
    return out


# revision 8
# speedup vs baseline: 1.5381x; 1.5381x over previous
"""Trainium2 Bass kernel for the controlled-unitary problem.

reference semantics (control=0, num_qubits=13, dim=8192):
    mask bit = 1 << 12, so columns/rows with that bit set are idx 4096..8191.
    out[:, c0] = state[:, c0]                       (control bit off: untouched)
    out[:, c1] = state[:, c1] @ target[c1, c1]      (controlled unitary)

Device work: complex [256,4096] @ [4096,4096] GEMM, Gauss 3-mult form.
Sharding: output columns of the GEMM split 8 ways (each core gets a
[4096, 512] slab of the target block; every weight byte moves once).

Per-core kernel (v3):
  - ONE packed input dram tensor x[128, KT, 2, 768] holding all four
    planes (a_r|b_r / a_i|b_i interleaved per k-tile) -> one dma_start
    per chunk, 3KB-per-partition descriptors, rings alternated.
  - Gauss prep on views: Vector computes a_s and negates a_i in place;
    GpSimd computes b_s and b_d (in place over b_i).
  - k1 matmuls first per chunk so the PE starts on a_s/b_r while the
    B-side preps still run; last chunk is m-major so m0's epilogue
    hides under m1's matmuls.
  - Epilogue adds read the two PSUM banks directly (one tensor_tensor
    per output plane), Vector and GpSimd in parallel, outputs DMA'd
    per m-tile on both rings.
"""

import os

import numpy as np

BATCH = 256
DIM = 8192
HALF = 4096
N_CORES = 8
NSH = HALF // N_CORES  # 512 output columns per core
KT = HALF // 128  # 32 k-tiles
MT = BATCH // 128  # 2 m-tiles
FW = BATCH + NSH  # 768 packed free width per (ktile, re/im)
CHUNKS = [1, 1, 2, 4, 8, 8, 8]  # k-tiles per DMA chunk (sums to KT)

DT_NAME = os.environ.get("KERNEL_DT", "float16")

_CACHE = {}


def _np_dtype(dt_name):
    if dt_name == "float16":
        return np.float16
    if dt_name == "bfloat16":
        import ml_dtypes

        return ml_dtypes.bfloat16
    return np.float32


def _build(dt_name):
    import concourse.mybir as mybir
    import concourse.tile as tile
    from concourse import bacc

    DT = getattr(mybir.dt, dt_name)
    F32 = mybir.dt.float32

    nc = bacc.Bacc("TRN2", target_bir_lowering=False, debug=False,
                   num_devices=N_CORES)

    x = nc.dram_tensor("x", [128, KT, 2, FW], DT, kind="ExternalInput")
    c = nc.dram_tensor("c", [2, BATCH, NSH], F32, kind="ExternalOutput")

    with tile.TileContext(nc) as tc:
        with (
            tc.tile_pool(name="xp", bufs=2) as xp,
            tc.tile_pool(name="dp", bufs=2) as dp,
            tc.tile_pool(name="op", bufs=1) as op,
            tc.tile_pool(name="ps", bufs=1, space="PSUM") as ps_pool,
        ):
            # Gauss 3-multiplication complex GEMM:
            #   k1 = (a_r+a_i).b_r   k2 = a_r.(b_i-b_r)   k3 = (-a_i).(b_r+b_i)
            #   C_r = k1 + k3        C_i = k1 + k2
            ps = {}
            for m in range(MT):
                for comp in ("t1", "t2", "t3"):
                    ps[(m, comp)] = ps_pool.tile(
                        [128, NSH], F32, name=f"ps_{m}_{comp}"
                    )

            rings = [nc.sync, nc.scalar]

            # PE warm-up: the HAM throttle holds the PE at 1.2 GHz for its
            # first ~3.4us of busy time.  The PE would otherwise idle while
            # the first DMA lands, so burn that window on dummy matmuls.
            warm = op.tile([128, NSH], DT, name="warm")
            wps = ps_pool.tile([128, NSH], F32, name="ps_warm")
            nc.gpsimd.memset(warm[:], 0)
            for _ in range(8):
                nc.tensor.matmul(wps[:], warm[:, :128], warm[:],
                                 start=True, stop=True)

            k0 = 0
            for ci, ch in enumerate(CHUNKS):
                nb = 3 if ch == 8 else 2
                x_t = xp.tile([128, ch, 2, FW], DT, name=f"x{ch}", bufs=nb)
                rings[ci % 2].dma_start(x_t[:], x[:, k0:k0 + ch, :, :])

                as_t = dp.tile([128, ch, BATCH], DT, name=f"as{ch}", bufs=nb)
                bs_t = dp.tile([128, ch, NSH], DT, name=f"bs{ch}", bufs=nb)
                # prep per k-tile so every DVE op sees contiguous APs:
                # Vector sums the A planes, GpSimd the B planes.
                for kk in range(ch):
                    nc.vector.tensor_tensor(
                        as_t[:, kk, :], x_t[:, kk, 0, :BATCH],
                        x_t[:, kk, 1, :BATCH], mybir.AluOpType.add)
                    nc.gpsimd.tensor_tensor(
                        bs_t[:, kk, :], x_t[:, kk, 0, BATCH:],
                        x_t[:, kk, 1, BATCH:], mybir.AluOpType.add)

                # t-scheme: t1 = Ar@Br, t2 = Ai@Bi, t3 = (Ar+Ai)@(Br+Bi)
                #   C_r = t1 - t2,  C_i = t3 - t1 - t2
                # t1/t2 read raw DMA data -> no prep on the PE critical path.
                def ops(comp, kk, msl):
                    if comp == "t1":
                        return x_t[:, kk, 0, msl.start:msl.stop], \
                            x_t[:, kk, 0, BATCH:]
                    if comp == "t2":
                        return x_t[:, kk, 1, msl.start:msl.stop], \
                            x_t[:, kk, 1, BATCH:]
                    return as_t[:, kk, msl], bs_t[:, kk, :]

                last_chunk = k0 + ch == KT
                if last_chunk:
                    # m-major: m0's epilogue hides under m1's matmuls
                    order = [(m, comp) for m in range(MT)
                             for comp in ("t1", "t2", "t3")]
                else:
                    order = [(m, comp) for comp in ("t1", "t2", "t3")
                             for m in range(MT)]
                for m, comp in order:
                    msl = slice(m * 128, (m + 1) * 128)
                    for kk in range(ch):
                        k = k0 + kk
                        lhs_v, rhs_v = ops(comp, kk, msl)
                        nc.tensor.matmul(
                            ps[(m, comp)][:], lhs_v, rhs_v, start=(k == 0),
                            stop=(last_chunk and kk == ch - 1),
                        )
                    if last_chunk and comp == "t1":
                        c1 = op.tile([128, NSH], F32, name=f"c1_{m}")
                        nc.scalar.activation(c1[:], ps[(m, "t1")][:],
                                             mybir.ActivationFunctionType.Copy)
                    if last_chunk and comp == "t2":
                        # out_r and u computed while t3 still streams
                        c2 = op.tile([128, NSH], F32, name=f"c2_{m}")
                        nc.scalar.activation(c2[:], ps[(m, "t2")][:],
                                             mybir.ActivationFunctionType.Copy)
                        out_r = op.tile([128, NSH], F32, name=f"or{m}")
                        u = op.tile([128, NSH], F32, name=f"u{m}")
                        nc.vector.tensor_tensor(
                            out_r[:], c1[:], c2[:], mybir.AluOpType.subtract)
                        nc.gpsimd.tensor_tensor(
                            u[:], c1[:], c2[:], mybir.AluOpType.add)
                        rings[m % 2].dma_start(c[0, msl, :], out_r[:])
                    if last_chunk and comp == "t3":
                        out_i = op.tile([128, NSH], F32, name=f"oi{m}")
                        nc.vector.tensor_tensor(
                            out_i[:], ps[(m, "t3")][:], u[:],
                            mybir.AluOpType.subtract)
                        rings[(m + 1) % 2].dma_start(c[1, msl, :], out_i[:])
                k0 += ch

    nc.compile()
    return nc


def _get_nc(dt_name):
    if dt_name not in _CACHE:
        _CACHE[dt_name] = _build(dt_name)
    return _CACHE[dt_name]


def _pack_inputs(A, B, np_dt):
    """A: [256, 4096] complex64, B: [4096, 4096] complex64 (full slab).
    Returns per-core packed x arrays [128, KT, 2, 768]."""
    at = A.T  # [4096, 256]
    # [4096, F] -> [128, KT, F] with k = kt*128 + p
    def kxm(m):
        f = m.shape[1]
        return m.reshape(KT, 128, f).transpose(1, 0, 2)

    a_r = kxm(np.ascontiguousarray(at.real))
    a_i = kxm(np.ascontiguousarray(at.imag))
    xs = []
    for cidx in range(N_CORES):
        csl = slice(cidx * NSH, (cidx + 1) * NSH)
        b_r = kxm(np.ascontiguousarray(B.real[:, csl]))
        b_i = kxm(np.ascontiguousarray(B.imag[:, csl]))
        xc = np.empty((128, KT, 2, FW), dtype=np_dt)
        xc[:, :, 0, :BATCH] = a_r
        xc[:, :, 0, BATCH:] = b_r
        xc[:, :, 1, :BATCH] = a_i
        xc[:, :, 1, BATCH:] = b_i
        xs.append(xc)
    return xs


def run_device(A, B, dt_name=DT_NAME, trace=False):
    """A: [256, 4096] complex64, B: [4096, 4096] complex64.
    Returns C = A @ B as [256, 4096] complex64 plus the raw results."""
    from concourse import bass_utils

    nc = _get_nc(dt_name)
    np_dt = _np_dtype(dt_name)

    xs = _pack_inputs(A, B, np_dt)
    in_maps = [{"x": xc} for xc in xs]

    res = bass_utils.run_bass_kernel_spmd(
        nc, in_maps, core_ids=list(range(N_CORES)), trace=trace
    )

    out = np.empty((BATCH, HALF), dtype=np.complex64)
    for cidx in range(N_CORES):
        csl = slice(cidx * NSH, (cidx + 1) * NSH)
        out.real[:, csl] = res.results[cidx]["c"][0]
        out.imag[:, csl] = res.results[cidx]["c"][1]
    return out, res


def kernel(state, target_matrix, control, num_qubits):
    state = np.asarray(state)
    target_matrix = np.asarray(target_matrix)
    control = int(control)
    num_qubits = int(num_qubits)
    dim = 1 << num_qubits

    assert state.shape == (BATCH, DIM) and dim == DIM, (
        "kernel hardcoded for [256, 8192]"
    )

    mask = 1 << (num_qubits - control - 1)
    idx = np.arange(dim)
    c1 = idx[(idx & mask) != 0]  # columns with control bit set

    if control == 0:
        A = state[:, HALF:]
        B = target_matrix[HALF:, HALF:]
    else:
        A = state[:, c1]
        B = target_matrix[np.ix_(c1, c1)]
    A = np.ascontiguousarray(A, dtype=np.complex64)
    B = np.ascontiguousarray(B, dtype=np.complex64)

    C, _ = run_device(A, B)

    out = state.astype(np.complex64, copy=True)
    out[:, c1] = C
    return out
